# revision 70
# baseline (speedup 1.0000x reference)
"""Trainium2 Bass kernel for nn_MultiHeadedAttention — fp8 DoubleRow v5.

Scores ride ONE fp8e4m3 DoubleRow matmul per (head, k-tile, q-half):
contraction K=65 x 2 = 130 slots hold the 4 cross products of a hi/lo fp8
split of both sides (x = x_hi + x_lo, each e4m3; ~0.1% rel err per entry)
plus the Schraudolph aux slot. Cost model: DoubleRow = 0.5 cycles per
output row vs 1 for f32r/bf16, and cost is independent of contraction
depth, so the hi/lo split and the aux row are free on the PE.

Slot map (partition p, DoubleRow half t):
  p in [0,32):  q side q_hi[d=p] (t-broadcast, stride-0), k side (k_hi, k_lo)
  p in [32,64): q side q_lo[d],                           k side (k_hi, k_lo)
  p = 64:       q side aux_q (both t),                    k side (1, 0)
The psum result is ps' = (A*(s - m) + B)/DIV in scaled Schraudolph space
(DIV=128 keeps aux inside e4m3 range; aux quantization shifts num and den
identically, so it cancels exactly in num/den).

exp + mask, alternating per half-tile (j):
  d: ONE DVE scalar_tensor_tensor: u16 = (ps' * DIV) * mask — saturating
     u16 cast clamps negatives to +0.0 bits, so the u16 output IS the bf16
     bit pattern of 2^((ps-B)/128) ~ exp(s-m) with mask folded in.
  a: ACT exp (scale=DIV/A undoes the prescale) -> Pool bf16 mask-mult
Softmax num/den ride the PE: [num; den] = [v|1]^T @ et, bf16, per k-tile.
num/den go to DRAM; division and head-mean are host-side.

The k and q projections run as ONE merged pipeline (no barrier: the q
chunks depend on k only through the tiny tsh reduction feeding the aux8
side branch). Projected directions stream through Pool hi/lo e4m3 casts
into partition-sliced relayout DMAs spread over the SP/ACT/Pool queues;
kdT8/qdT8 are split into half tiles so relayouts land incrementally.
Biases ride the ACT activations as per-partition operands instead of
rank-1 bias matmuls. Projection inputs (qT/kT/w0/w1) ship as bf16: the
matmuls cost the same (1 cyc/row), xT SBUF and DMA bytes halve, and the
freed SBUF double-buffers the per-chunk chain temps (sq/rw/sqn/srt/rn)
for deeper cross-chunk pipelining.
601890 ns (f32r baseline) -> 437313 ns cost-model, rel err 2.2e-3.
Sharding: core c -> batch b=c//2, query-half c%2.
"""

import numpy as np

import concourse.mybir as mybir
from concourse import bacc
from concourse.tile import TileContext
from concourse import bass_utils

F32 = mybir.dt.float32
F32R = mybir.dt.float32r
BF16 = mybir.dt.bfloat16
F8 = mybir.dt.float8e4
U16 = mybir.dt.uint16

B, SQ, SK, D, H, DK = 4, 4096, 4096, 256, 8, 32
NCORES = 8
R = SQ // 2          # q rows per core
QH = R // 1024       # 2 q-half blocks of 1024
KT = SK // 128       # 32 k-tiles of 128
SCALE = 10.0 / (32.0 ** 0.25)
LAM = 1.51           # shift coefficient, window [1.36, 1.66]
A16 = 128.0 / np.log(2.0)          # schraudolph scale (bf16-bits space)
C16 = -7.5                         # schraudolph bias correction
B16 = 127.0 * 128.0 + C16
DIV = 128.0                        # fp8-space divisor: ps' = ps/DIV
ALPHA = float(np.sqrt((A16 / DIV) * SCALE * SCALE))  # per-side dir scale

ROTS = [(0, 1), (2, 3), (4, 5), (6, 7)]
# d:a = 63:65 of 128 — rebalances DVE (658ns/half) vs ACT (619ns/half +
# the halved ndsb evacuation load)
ROUTE = ['d' if (i % 2 == 0 and i != 64) else 'a' for i in range(128)]

_CACHE = {}


def _build(repeat=1):
    if repeat in _CACHE:
        return _CACHE[repeat]
    nc = bacc.Bacc("TRN2", target_bir_lowering=False, debug=False,
                   num_devices=NCORES)

    qT_d = nc.dram_tensor("qT", [D, R], BF16, kind="ExternalInput")
    kT_d = nc.dram_tensor("kT", [D, SK], BF16, kind="ExternalInput")
    v_d = nc.dram_tensor("v", [1, SK], F32, kind="ExternalInput")
    mt_d = nc.dram_tensor("mt", [SK, R], BF16, kind="ExternalInput")
    w0p_d = nc.dram_tensor("w0p", [D, 4 * 128], BF16, kind="ExternalInput")
    w1t8_d = nc.dram_tensor("w1t8", [D, H], BF16, kind="ExternalInput")
    b0c_d = nc.dram_tensor("b0c", [128, 4], F32, kind="ExternalInput")
    b18c_d = nc.dram_tensor("b18c", [8, 2], F32, kind="ExternalInput")
    inds_d = nc.dram_tensor("inds", [128, 4 * H], F32, kind="ExternalInput")
    indst_d = nc.dram_tensor("indst", [H, 4 * 128], F32, kind="ExternalInput")
    kaux_d = nc.dram_tensor("kaux", [1, H * SK // 2], F8,
                            kind="ExternalInput")
    out_d = nc.dram_tensor("o", [8, QH * 2048], F32, kind="ExternalOutput")

    def mm(out, lhsT, rhs, **kw):
        nc.tensor.matmul(out, lhsT.bitcast(F32R), rhs.bitcast(F32R), **kw)

    phases = [(rep, qh) for rep in range(repeat) for qh in range(QH)]

    with TileContext(nc) as tc:
        with tc.tile_pool(name="persist", bufs=1) as pp, \
             tc.tile_pool(name="maskpA", bufs=1) as maskpA:
            w1t8 = pp.tile([128, 2, H], BF16, tag="w1t8")
            nc.gpsimd.dma_start(w1t8[:],
                                w1t8_d.rearrange("(a p) o -> p a o", p=128))
            b18c = pp.tile([8, 2], F32, tag="b18c")  # col0: b1, col1: SCALE*b1
            nc.gpsimd.dma_start(b18c[:].bitcast(F32R), b18c_d[:].bitcast(F32R))
            b0c = pp.tile([128, 4], F32, tag="b0c")
            nc.gpsimd.dma_start(b0c[:].bitcast(F32R), b0c_d[:].bitcast(F32R))
            expbias = pp.tile([128, 1], F32, tag="expbias")
            nc.gpsimd.memset(expbias[:], -B16 / A16)

            # fp8 score operands (DoubleRow layout, see module docstring),
            # split by k-half / q-half for early main-loop start
            kdT8 = [pp.tile([65, 2, H, SK // 2], F8, tag=f"kdT8{i}",
                            name=f"kdT8{i}") for i in range(2)]
            qdT8 = [pp.tile([65, H, 1024], F8, tag=f"qdT8{i}",
                            name=f"qdT8{i}") for i in range(QH)]
            # aux k-side slot: (64, t0) = 1.0 via DRAM constant (a memset of
            # a single-partition row costs ~27us of engine time); (64, t1)=0
            # comes from the lo-relayout DMA reading one zero-padding row.

            # mask quarters: tag j covers k-tiles 8j..8j+7 of one query-half.
            mask_tiles = {}
            mask_pools = {0: maskpA}

            def ensure_mask(ph, j):
                if ph >= len(phases) or (ph, j) in mask_tiles:
                    return
                rep, qh = phases[ph]
                q0 = qh * 1024
                t = mask_pools[j].tile([128, 8, 1024], BF16, tag=f"mq{j}")
                k0 = j * 8 * 128
                nc.sync.dma_start(
                    t[:],
                    mt_d[k0:k0 + 1024, q0:q0 + 1024].rearrange(
                        "(c p) q -> p c q", p=128))
                mask_tiles[(ph, j)] = t

            shp_ctx = tc.tile_pool(name="shp", bufs=1)
            shp = shp_ctx.__enter__()
            sskp = shp.tile([8, 8], F32, tag="sskp")     # per-chunk sum kn'^2
            ssk = shp.tile([8, 1], F32, tag="ssk")
            tsh = shp.tile([8, 1], F32, tag="tsh")
            tshA = shp.tile([8, 1], F32, tag="tshA")     # tsh * -A16/DIV
            mq = shp.tile([8, R], BF16, tag="mq")        # SCALE*|qn|
            aux8 = shp.tile([8, R], F8, tag="aux8")
            uvt = pp.tile([128, KT, 2], BF16, tag="uvt")
            nc.gpsimd.dma_start(uvt[:, :, 0],
                                v_d.rearrange("a (c p) -> p (a c)", p=128))
            nc.gpsimd.memset(uvt[:, :, 1:2], 1.0)

            # ---- merged k+q projection: one pipeline, no barrier. The q
            # chunks only depend on k through the tiny tsh reduction, which
            # feeds the aux8 side branch, not the main direction chain.
            with (
                tc.tile_pool(name="xTp", bufs=1) as xTp,
                tc.tile_pool(name="psP", bufs=3, space="PSUM") as psP,
                tc.tile_pool(name="psN", bufs=2, space="PSUM") as psN,
                tc.tile_pool(name="psS", bufs=1, space="PSUM") as psS,
                tc.tile_pool(name="psE", bufs=2, space="PSUM") as psE,
                tc.tile_pool(name="sqp", bufs=2) as sqp,
                tc.tile_pool(name="smp", bufs=2) as smp,
                tc.tile_pool(name="sm1", bufs=1) as sm1,
                tc.tile_pool(name="xdp", bufs=2) as xdp,
                tc.tile_pool(name="hilo", bufs=1) as hlp,
                tc.tile_pool(name="cst", bufs=1) as cstp,
            ):
                w0p = cstp.tile([128, 2, 4, 128], BF16, tag="w0p")
                nc.sync.dma_start(
                    w0p[:],
                    w0p_d.rearrange("(a p) (g o) -> p a g o", p=128, g=4))
                inds = cstp.tile([128, 4, H], BF16, tag="inds")
                nc.gpsimd.dma_start(
                    inds[:], inds_d.rearrange("p (g o) -> p g o", g=4))
                indst = cstp.tile([H, 4, 128], F32, tag="indst")
                nc.gpsimd.dma_start(
                    indst[:].bitcast(F32R),
                    indst_d.rearrange("p (g o) -> p g o", g=4).bitcast(F32R))
                xTk = xTp.tile([128, 2, SK], BF16, tag="xTk")
                for xh in range(4):
                    nc.sync.dma_start(
                        xTk[:, :, xh * 1024:(xh + 1) * 1024],
                        kT_d[:, xh * 1024:(xh + 1) * 1024].rearrange(
                            "(kc p) r -> p kc r", p=128))

                xTq = xTp.tile([128, 2, R], BF16, tag="xTq")
                for xh in range(2):
                    nc.sync.dma_start(
                        xTq[:, :, xh * 1024:(xh + 1) * 1024],
                        qT_d[:, xh * 1024:(xh + 1) * 1024].rearrange(
                            "(kc p) r -> p kc r", p=128))

                hi8 = lo8 = None

                def relayout(is_q, stg):
                    if is_q:
                        dst = qdT8[stg]
                        for u in range(2):
                            s = slice(64 * u, 64 * u + 32)
                            nc.scalar.dma_start(dst[0:32, u::2, :],
                                                hi8[s])
                            nc.gpsimd.dma_start(dst[32:64, u::2, :],
                                                lo8[s])

                    else:
                        dst = kdT8[stg]
                        for u in range(2):
                            s = slice(64 * u, 64 * u + 32)
                            s33 = slice(64 * u, 64 * u + 33)
                            nc.scalar.dma_start(
                                dst[0:32, 0, u::2, :], hi8[s])
                            nc.gpsimd.dma_start(
                                dst[32:64, 0, u::2, :], hi8[s])
                            nc.sync.dma_start(
                                dst[0:32, 1, u::2, :], lo8[s])
                            # 33 rows: row 64u+32 is zero padding and lands
                            # on the aux slot (64, t1) = 0
                            nc.sync.dma_start(
                                dst[32:65, 1, u::2, :], lo8[s33])

                sched = [(False, 0), (False, 1), (True, 0), (False, 2),
                         (False, 3), (True, 1), (False, 4), (False, 5),
                         (True, 2), (False, 6), (False, 7), (True, 3)]
                stg_tiles = {}
                for is_q, ch in sched:
                    spc = 2 if is_q else 4
                    xT = xTq if is_q else xTk
                    if True:
                        cs = slice(ch * 512, (ch + 1) * 512)
                        css = slice((ch % spc) * 512, (ch % spc + 1) * 512)
                        pfx = "q" if is_q else "k"
                        if ch % spc == 0:
                            stg_tiles[pfx] = (
                                hlp.tile([128, 4, spc * 512], F8,
                                         tag=pfx + "hi8",
                                         name=f"hi8{is_q}{ch}"),
                                hlp.tile([128, 4, spc * 512], F8,
                                         tag=pfx + "lo8",
                                         name=f"lo8{is_q}{ch}"))
                        hi8, lo8 = stg_tiles[pfx]
                        if not is_q and ch == 3:
                            ensure_mask(0, 0)
                        if not is_q and ch == 4:
                            # aux k-row ones, issued mid-projection so the
                            # ACT queue serves it in an engine gap
                            for i in range(2):
                                nc.sync.dma_start(
                                    kdT8[i][64:65, 0].rearrange(
                                        "a h k -> a (h k)"), kaux_d[:])
                        # norms projection qn[8, 512] (bias via ACT)
                        pn = psN.tile([8, 512], F32, tag="pn")
                        for kc in range(2):
                            nc.tensor.matmul(
                                pn[:], w1t8[:, kc, :], xT[:, kc, cs],
                                start=(kc == 0), stop=(kc == 1))
                        if is_q:
                            # mq = SCALE*|qn| = |SCALE*pn + SCALE*b1|
                            nc.scalar.activation(
                                mq[:, cs], pn[:],
                                mybir.ActivationFunctionType.Abs,
                                bias=b18c[:, 1:2], scale=SCALE)
                        else:
                            sqn = smp.tile([8, 512], F32, tag="sqn")
                            nc.scalar.activation(
                                sqn[:], pn[:],
                                mybir.ActivationFunctionType.Square,
                                bias=b18c[:, 0:1], scale=1.0)
                            nc.vector.tensor_reduce(
                                sskp[:, ch:ch + 1], sqn[:],
                                axis=mybir.AxisListType.X,
                                op=mybir.AluOpType.add)
                        # per-group direction projections + scaling;
                        # rw = pr + b0 (an ACT/DVE op each — a fused variant
                        # reading pr and pe together is illegal: one PSUM
                        # input max per vector instruction, NCC_IBVF027)
                        sq_ = [None] * 4
                        rw_ = [None] * 4
                        for gp in range(4):
                            pr = psP.tile([128, 512], F32, tag="pr",
                                          name=f"pr{gp}")
                            for kc in range(2):
                                nc.tensor.matmul(
                                    pr[:], w0p[:, kc, gp, :], xT[:, kc, cs],
                                    start=(kc == 0), stop=(kc == 1))
                            sq_[gp] = sqp.tile([128, 512], BF16,
                                               tag=f"sq{gp}", name=f"sq{gp}")
                            nc.scalar.activation(
                                sq_[gp][:], pr[:],
                                mybir.ActivationFunctionType.Square,
                                bias=b0c[:, gp:gp + 1], scale=1.0)
                            rw_[gp] = sqp.tile([128, 512], F32,
                                               tag=f"rw{gp}", name=f"rw{gp}")
                            if gp < 2:
                                nc.scalar.activation(
                                    rw_[gp][:], pr[:],
                                    mybir.ActivationFunctionType.Identity,
                                    bias=b0c[:, gp:gp + 1], scale=1.0)
                            else:
                                nc.vector.tensor_scalar(
                                    out=rw_[gp][:], in0=pr[:],
                                    scalar1=b0c[:, gp:gp + 1], scalar2=0.0,
                                    op0=mybir.AluOpType.add,
                                    op1=mybir.AluOpType.add)
                        pss = psS.tile([8, 512], F32, tag="pss")
                        for gp in range(4):
                            nc.tensor.matmul(pss[:], inds[:, gp, :],
                                             sq_[gp][:],
                                             start=(gp == 0), stop=(gp == 3))
                        srt = smp.tile([8, 512], F32, tag="srt")
                        nc.scalar.activation(
                            srt[:], pss[:],
                            mybir.ActivationFunctionType.Sqrt,
                            scale=1.0 / (ALPHA * ALPHA))
                        rn = smp.tile([8, 512], F32, tag="rn")
                        nc.vector.reciprocal_approx_fast(rn[:], srt[:])
                        av = smp.tile([8, 512], F32, tag="av")
                        nc.vector.scalar_tensor_tensor(
                            out=av[:].bitcast(F32R), in0=pn[:],
                            scalar=b18c[:, 0:1], in1=rn[:],
                            op0=mybir.AluOpType.add,
                            op1=mybir.AluOpType.mult)

                        xd = xdp.tile([128, 4, 512], BF16, tag="xd")
                        for gp in range(4):
                            pe = psE.tile([128, 512], F32, tag="pe")
                            mm(pe[:], indst[:, gp, :], av[:],
                               start=True, stop=True)
                            nc.vector.tensor_mul(
                                xd[:, gp], rw_[gp][:], pe[:])
                        # hi/lo e4m3 split on Pool
                        nc.gpsimd.tensor_scalar(
                            out=hi8[:, :, css], in0=xd[:], scalar1=1.0,
                            scalar2=0.0, op0=mybir.AluOpType.mult,
                            op1=mybir.AluOpType.add)
                        nc.gpsimd.tensor_tensor(
                            out=lo8[:, :, css], in0=xd[:],
                            in1=hi8[:, :, css],
                            op=mybir.AluOpType.subtract)
                        if ch % spc == spc - 1:
                            relayout(is_q, ch // spc)
                # shift scale + deferred aux rows (they need the full-k tsh)
                nc.vector.tensor_reduce(ssk[:], sskp[:],
                                        axis=mybir.AxisListType.X,
                                        op=mybir.AluOpType.add)
                nc.scalar.activation(tsh[:], ssk[:],
                                     mybir.ActivationFunctionType.Sqrt,
                                     scale=LAM * LAM * SCALE * SCALE
                                     / float(SK))
                nc.scalar.mul(tshA[:], tsh[:], -A16 / DIV)
                for stg in range(QH):
                    ss = slice(stg * 1024, (stg + 1) * 1024)
                    nc.vector.tensor_scalar(
                        out=aux8[:, ss], in0=mq[:, ss],
                        scalar1=tshA[:], scalar2=B16 / DIV,
                        op0=mybir.AluOpType.mult, op1=mybir.AluOpType.add)
                    for h in range(H):
                        nc.sync.dma_start(
                            qdT8[stg][64:65, h, :],
                            aux8[h:h + 1, ss])

            shp_ctx.__exit__(None, None, None)

            # ---- main attention loop ----
            with (
                tc.tile_pool(name="maskpB", bufs=1) as maskpB,
                tc.tile_pool(name="psSc", bufs=6, space="PSUM") as psc,
                tc.tile_pool(name="psNd", bufs=1, space="PSUM") as psnd,
                tc.tile_pool(name="eraw", bufs=6) as erawp,
                tc.tile_pool(name="etl", bufs=12) as etlp,
                tc.tile_pool(name="ndsb", bufs=2) as ndsbp,
            ):
                for _j in (1, 2, 3):
                    mask_pools[_j] = maskpB
                for ph, (rep, qh) in enumerate(phases):
                    for j in range(4):
                        ensure_mask(ph, j)
                    for ri, rot in enumerate(ROTS):
                        if ri % 2 == 0:
                            ndall = psnd.tile([98, 1024], F32, tag="ndall")
                        nb = 64 * (ri % 2)
                        last_rot = ri == len(ROTS) - 1
                        for kc in range(KT):
                            msl = mask_tiles[(ph, kc // 8)][:, kc % 8, :]
                            for slot, h in enumerate(rot):
                                co = nb + 32 * slot
                                kv = kdT8[kc // 16][
                                    0:65, :, h,
                                    (kc % 16) * 128:(kc % 16 + 1) * 128]
                                for j in range(2):
                                    js = slice(j * 512, (j + 1) * 512)
                                    ps = psc.tile([128, 512], F32, tag="ps",
                                                  name=f"ps{slot}{j}")
                                    qv = qdT8[qh][0:65, h,
                                                  j * 512:(j + 1) * 512]
                                    nc.tensor.matmul(
                                        ps[:], kv,
                                        qv.unsqueeze(1).broadcast_to(
                                            [65, 2, 512]),
                                        start=True, stop=True,
                                        perf_mode=(
                                            mybir.MatmulPerfMode.DoubleRow),
                                        tile_position=(0, 0))
                                    et = etlp.tile([128, 512], BF16,
                                                   tag="et")
                                    if ROUTE[(kc * 4 + slot * 2 + j)
                                             % 128] == 'd':
                                        # d: fused DVE schraudolph (u16 sat)
                                        nc.vector.scalar_tensor_tensor(
                                            out=et[:].bitcast(U16),
                                            in0=ps[:], scalar=DIV,
                                            in1=msl[:, js],
                                            op0=mybir.AluOpType.mult,
                                            op1=mybir.AluOpType.mult)
                                    else:
                                        # a: ACT exp -> Pool mask-mult
                                        er = erawp.tile([128, 512], BF16,
                                                        tag="er")
                                        nc.scalar.activation(
                                            er[:], ps[:],
                                            mybir.ActivationFunctionType.Exp,
                                            bias=expbias[:], scale=DIV / A16)
                                        nc.gpsimd.tensor_tensor(
                                            out=et[:], in0=er[:],
                                            in1=msl[:, js],
                                            op=mybir.AluOpType.mult)
                                    nc.tensor.matmul(
                                        ndall[co:co + 2,
                                              j * 512:(j + 1) * 512],
                                        uvt[:, kc, :], et[:],
                                        start=(kc == 0), stop=(kc == KT - 1),
                                        tile_position=(0, co))
                            if last_rot and kc % 8 == 7:
                                ensure_mask(ph + 1, kc // 8)
                        if ri % 2 == 1:
                            # one evacuation per rotation pair: the copy is
                            # charged by free size, so 98 rows cost the same
                            # as 34
                            ndsb = ndsbp.tile([98, 1024], F32, tag="ndsb")
                            nc.scalar.copy(ndsb[:], ndall[0:98, :])
                            for o, orot in enumerate(ROTS[ri - 1:ri + 1]):
                                for slot, h in enumerate(orot):
                                    r0 = 64 * o + 32 * slot
                                    nc.sync.dma_start(
                                        out_d[h:h + 1,
                                              qh * 2048:(qh + 1) * 2048],
                                        ndsb[r0:r0 + 2, :])

    nc.finalize()
    _CACHE[repeat] = nc
    return nc


def _prep_host(query, key, value, mask, w0, b0, w1, b1):
    import ml_dtypes
    # outc permutation: group gp = h//2 holds head 2gp at rows 0-31 and head
    # 2gp+1 at rows 64-95; rows 32-63/96-127 are zero padding.
    w0p = np.zeros((D, 4 * 128), np.float32)
    b0c = np.zeros((128, 4), np.float32)
    inds = np.zeros((128, 4 * H), np.float32)
    indst = np.zeros((H, 4 * 128), np.float32)
    w0t = w0.T.astype(np.float32)            # [inc, outc]
    for h in range(H):
        gp, u = divmod(h, 2)
        dst = gp * 128 + 64 * u
        w0p[:, dst:dst + 32] = w0t[:, 32 * h:32 * h + 32]
        b0c[64 * u:64 * u + 32, gp] = b0[32 * h:32 * h + 32]
        inds[64 * u:64 * u + 32, gp * H + h] = 1.0
        indst[h, gp * 128 + 64 * u:gp * 128 + 64 * u + 32] = 1.0
    w1t8 = np.ascontiguousarray(w1[:H].T).astype(ml_dtypes.bfloat16)
    b18c = np.stack([b1[:H], SCALE * b1[:H]], axis=1).astype(np.float32)
    w0p = w0p.astype(ml_dtypes.bfloat16)
    kaux = np.ones((1, H * SK // 2), ml_dtypes.float8_e4m3)
    in_maps = []
    for c in range(NCORES):
        b, half = divmod(c, 2)
        r0 = half * R
        mt = np.ascontiguousarray(mask[b, r0:r0 + R].T).astype(
            ml_dtypes.bfloat16)
        in_maps.append({
            "qT": np.ascontiguousarray(query[b, r0:r0 + R].T).astype(
                ml_dtypes.bfloat16),
            "kT": np.ascontiguousarray(key[b].T).astype(ml_dtypes.bfloat16),
            "v": np.ascontiguousarray(value[b].reshape(1, SK)),
            "mt": mt,
            "w0p": w0p, "w1t8": w1t8, "b0c": b0c, "b18c": b18c,
            "inds": inds, "indst": indst, "kaux": kaux,
        })
    return in_maps


def kernel(query, key, value, mask, w0, b0, w1, b1, _repeat=1):
    query = np.asarray(query, np.float32)
    key = np.asarray(key, np.float32)
    value = np.asarray(value, np.float32)
    mask = np.asarray(mask, np.int32)
    nc = _build(_repeat)
    in_maps = _prep_host(query, key, value, mask, w0, b0, w1, b1)
    res = bass_utils.run_bass_kernel_spmd(nc, in_maps,
                                          core_ids=list(range(NCORES)))
    out = np.empty((B, SQ, 1), np.float32)
    for c in range(NCORES):
        b, half = divmod(c, 2)
        o = res.results[c]["o"].reshape(8, QH, 2048)
        x = o[:, :, 0:1024] / o[:, :, 1024:2048]
        out[b, half * R:(half + 1) * R, 0] = x.reshape(8, R).mean(axis=0)
    return out


# revision 71
# speedup vs baseline: 1.0267x; 1.0267x over previous
"""Trainium2 Bass kernel for nn_MultiHeadedAttention — fp8 DoubleRow v5.

Scores ride ONE fp8e4m3 DoubleRow matmul per (head, k-tile, q-half):
contraction K=65 x 2 = 130 slots hold the 4 cross products of a hi/lo fp8
split of both sides (x = x_hi + x_lo, each e4m3; ~0.1% rel err per entry)
plus the Schraudolph aux slot. Cost model: DoubleRow = 0.5 cycles per
output row vs 1 for f32r/bf16, and cost is independent of contraction
depth, so the hi/lo split and the aux row are free on the PE.

Slot map (partition p, DoubleRow half t):
  p in [0,32):  q side q_hi[d=p] (t-broadcast, stride-0), k side (k_hi, k_lo)
  p in [32,64): q side q_lo[d],                           k side (k_hi, k_lo)
  p = 64:       q side aux_q (both t),                    k side (1, 0)
The psum result is ps' = (A*(s - m) + B)/DIV in scaled Schraudolph space
(DIV=128 keeps aux inside e4m3 range; aux quantization shifts num and den
identically, so it cancels exactly in num/den).

exp + mask, alternating per half-tile (j):
  d: ONE DVE scalar_tensor_tensor: u16 = (ps' * DIV) * mask — saturating
     u16 cast clamps negatives to +0.0 bits, so the u16 output IS the bf16
     bit pattern of 2^((ps-B)/128) ~ exp(s-m) with mask folded in.
  a: ACT exp (scale=DIV/A undoes the prescale) -> Pool bf16 mask-mult
Softmax num/den ride the PE: [num; den] = [v|1]^T @ et, bf16, per k-tile.
num/den go to DRAM; division and head-mean are host-side.

The k and q projections run as ONE merged pipeline (no barrier: the q
chunks depend on k only through the tiny tsh reduction feeding the aux8
side branch). Projected directions stream through Pool hi/lo e4m3 casts
into partition-sliced relayout DMAs spread over the SP/ACT/Pool queues;
kdT8/qdT8 are split into half tiles so relayouts land incrementally.
Biases ride the ACT activations as per-partition operands instead of
rank-1 bias matmuls. Projection inputs (qT/kT/w0/w1) ship as bf16: the
matmuls cost the same (1 cyc/row), xT SBUF and DMA bytes halve, and the
freed SBUF double-buffers the per-chunk chain temps (sq/rw/sqn/srt/rn)
for deeper cross-chunk pipelining.
601890 ns (f32r baseline) -> 437313 ns cost-model, rel err 2.2e-3.
Sharding: core c -> batch b=c//2, query-half c%2.
"""

import numpy as np

import concourse.mybir as mybir
from concourse import bacc
from concourse.tile import TileContext
from concourse import bass_utils

F32 = mybir.dt.float32
F32R = mybir.dt.float32r
BF16 = mybir.dt.bfloat16
F8 = mybir.dt.float8e4
U16 = mybir.dt.uint16

B, SQ, SK, D, H, DK = 4, 4096, 4096, 256, 8, 32
NCORES = 8
R = SQ // 2          # q rows per core
QH = R // 1024       # 2 q-half blocks of 1024
KT = SK // 128       # 32 k-tiles of 128
SCALE = 10.0 / (32.0 ** 0.25)
LAM = 1.51           # shift coefficient, window [1.36, 1.66]
A16 = 128.0 / np.log(2.0)          # schraudolph scale (bf16-bits space)
C16 = -7.5                         # schraudolph bias correction
B16 = 127.0 * 128.0 + C16
DIV = 128.0                        # fp8-space divisor: ps' = ps/DIV
ALPHA = float(np.sqrt((A16 / DIV) * SCALE * SCALE))  # per-side dir scale

ROTS = [(0, 1), (2, 3), (4, 5), (6, 7)]
# d:a = 63:65 of 128 — rebalances DVE (658ns/half) vs ACT (619ns/half +
# the halved ndsb evacuation load)
ROUTE = ['d' if (i % 2 == 0 and i != 64) else 'a' for i in range(128)]

_CACHE = {}


def _build(repeat=1):
    if repeat in _CACHE:
        return _CACHE[repeat]
    nc = bacc.Bacc("TRN2", target_bir_lowering=False, debug=False,
                   num_devices=NCORES)

    qT_d = nc.dram_tensor("qT", [D, R], BF16, kind="ExternalInput")
    kT_d = nc.dram_tensor("kT", [D, SK], BF16, kind="ExternalInput")
    v_d = nc.dram_tensor("v", [1, SK], F32, kind="ExternalInput")
    mt_d = nc.dram_tensor("mt", [SK, R], BF16, kind="ExternalInput")
    w0p_d = nc.dram_tensor("w0p", [D, 4 * 128], BF16, kind="ExternalInput")
    w1t8_d = nc.dram_tensor("w1t8", [D, H], BF16, kind="ExternalInput")
    b0c_d = nc.dram_tensor("b0c", [128, 4], F32, kind="ExternalInput")
    b18c_d = nc.dram_tensor("b18c", [8, 2], F32, kind="ExternalInput")
    inds_d = nc.dram_tensor("inds", [128, 4 * H], F32, kind="ExternalInput")
    indst_d = nc.dram_tensor("indst", [H, 4 * 128], F32, kind="ExternalInput")
    kaux_d = nc.dram_tensor("kaux", [1, H * SK // 2], F8,
                            kind="ExternalInput")
    out_d = nc.dram_tensor("o", [8, QH * 2048], F32, kind="ExternalOutput")

    def mm(out, lhsT, rhs, **kw):
        nc.tensor.matmul(out, lhsT.bitcast(F32R), rhs.bitcast(F32R), **kw)

    phases = [(rep, qh) for rep in range(repeat) for qh in range(QH)]

    with TileContext(nc) as tc:
        with tc.tile_pool(name="persist", bufs=1) as pp, \
             tc.tile_pool(name="maskpA", bufs=1) as maskpA:
            w1t8 = pp.tile([128, 2, H], BF16, tag="w1t8")
            nc.gpsimd.dma_start(w1t8[:],
                                w1t8_d.rearrange("(a p) o -> p a o", p=128))
            b18c = pp.tile([8, 2], F32, tag="b18c")  # col0: b1, col1: SCALE*b1
            nc.gpsimd.dma_start(b18c[:].bitcast(F32R), b18c_d[:].bitcast(F32R))
            b0c = pp.tile([128, 4], F32, tag="b0c")
            nc.gpsimd.dma_start(b0c[:].bitcast(F32R), b0c_d[:].bitcast(F32R))
            expbias = pp.tile([128, 1], F32, tag="expbias")
            nc.gpsimd.memset(expbias[:], -B16 / A16)

            # fp8 score operands (DoubleRow layout, see module docstring),
            # split by k-half / q-half for early main-loop start
            kdT8 = [pp.tile([65, 2, H, SK // 2], F8, tag=f"kdT8{i}",
                            name=f"kdT8{i}") for i in range(2)]
            qdT8 = [pp.tile([65, H, 1024], F8, tag=f"qdT8{i}",
                            name=f"qdT8{i}") for i in range(QH)]
            # aux k-side slot: (64, t0) = 1.0 via DRAM constant (a memset of
            # a single-partition row costs ~27us of engine time); (64, t1)=0
            # comes from the lo-relayout DMA reading one zero-padding row.

            # mask quarters: tag j covers k-tiles 8j..8j+7 of one query-half.
            mask_tiles = {}
            mask_pools = {0: maskpA}

            def ensure_mask(ph, j):
                if ph >= len(phases) or (ph, j) in mask_tiles:
                    return
                rep, qh = phases[ph]
                q0 = qh * 1024
                t = mask_pools[j].tile([128, 8, 1024], BF16, tag=f"mq{j}")
                k0 = j * 8 * 128
                nc.sync.dma_start(
                    t[:],
                    mt_d[k0:k0 + 1024, q0:q0 + 1024].rearrange(
                        "(c p) q -> p c q", p=128))
                mask_tiles[(ph, j)] = t

            shp_ctx = tc.tile_pool(name="shp", bufs=1)
            shp = shp_ctx.__enter__()
            sskp = shp.tile([8, 8], F32, tag="sskp")     # per-chunk sum kn'^2
            ssk = shp.tile([8, 1], F32, tag="ssk")
            tsh = shp.tile([8, 1], F32, tag="tsh")
            tshA = shp.tile([8, 1], F32, tag="tshA")     # tsh * -A16/DIV
            mq = shp.tile([8, R], BF16, tag="mq")        # SCALE*|qn|
            aux8 = shp.tile([8, R], F8, tag="aux8")
            uvt = pp.tile([128, KT, 2], BF16, tag="uvt")
            nc.gpsimd.dma_start(uvt[:, :, 0],
                                v_d.rearrange("a (c p) -> p (a c)", p=128))
            nc.gpsimd.memset(uvt[:, :, 1:2], 1.0)

            # ---- merged k+q projection: one pipeline, no barrier. The q
            # chunks only depend on k through the tiny tsh reduction, which
            # feeds the aux8 side branch, not the main direction chain.
            with (
                tc.tile_pool(name="xTp", bufs=1) as xTp,
                tc.tile_pool(name="psP", bufs=3, space="PSUM") as psP,
                tc.tile_pool(name="psN", bufs=2, space="PSUM") as psN,
                tc.tile_pool(name="psS", bufs=1, space="PSUM") as psS,
                tc.tile_pool(name="psE", bufs=2, space="PSUM") as psE,
                tc.tile_pool(name="sqp", bufs=2) as sqp,
                tc.tile_pool(name="smp", bufs=2) as smp,
                tc.tile_pool(name="sm1", bufs=1) as sm1,
                tc.tile_pool(name="xdp", bufs=2) as xdp,
                tc.tile_pool(name="hilo", bufs=1) as hlp,
                tc.tile_pool(name="cst", bufs=1) as cstp,
            ):
                w0p = cstp.tile([128, 2, 4, 128], BF16, tag="w0p")
                nc.sync.dma_start(
                    w0p[:],
                    w0p_d.rearrange("(a p) (g o) -> p a g o", p=128, g=4))
                inds = cstp.tile([128, 4, H], BF16, tag="inds")
                nc.gpsimd.dma_start(
                    inds[:], inds_d.rearrange("p (g o) -> p g o", g=4))
                indst = cstp.tile([H, 4, 128], F32, tag="indst")
                nc.gpsimd.dma_start(
                    indst[:].bitcast(F32R),
                    indst_d.rearrange("p (g o) -> p g o", g=4).bitcast(F32R))
                xTk = xTp.tile([128, 2, SK], BF16, tag="xTk")
                for xh in range(4):
                    nc.sync.dma_start(
                        xTk[:, :, xh * 1024:(xh + 1) * 1024],
                        kT_d[:, xh * 1024:(xh + 1) * 1024].rearrange(
                            "(kc p) r -> p kc r", p=128))

                xTq = xTp.tile([128, 2, R], BF16, tag="xTq")
                for xh in range(2):
                    nc.sync.dma_start(
                        xTq[:, :, xh * 1024:(xh + 1) * 1024],
                        qT_d[:, xh * 1024:(xh + 1) * 1024].rearrange(
                            "(kc p) r -> p kc r", p=128))

                hi8 = lo8 = None

                def relayout(is_q, stg):
                    if is_q:
                        dst = qdT8[stg]
                        for u in range(2):
                            s = slice(64 * u, 64 * u + 32)
                            nc.scalar.dma_start(dst[0:32, u::2, :],
                                                hi8[s, :, 0:1024])
                            nc.gpsimd.dma_start(dst[32:64, u::2, :],
                                                lo8[s, :, 0:1024])
                        for h in range(H):
                            nc.sync.dma_start(
                                dst[64:65, h, :],
                                aux8[h:h + 1,
                                     stg * 1024:(stg + 1) * 1024])
                    else:
                        dst = kdT8[stg]
                        for u in range(2):
                            s = slice(64 * u, 64 * u + 32)
                            s33 = slice(64 * u, 64 * u + 33)
                            nc.scalar.dma_start(
                                dst[0:32, 0, u::2, :], hi8[s])
                            nc.gpsimd.dma_start(
                                dst[32:64, 0, u::2, :], hi8[s])
                            nc.sync.dma_start(
                                dst[0:32, 1, u::2, :], lo8[s])
                            # 33 rows: row 64u+32 is zero padding and lands
                            # on the aux slot (64, t1) = 0
                            nc.sync.dma_start(
                                dst[32:65, 1, u::2, :], lo8[s33])

                segs = [(False, 8, 4, xTk), (True, 4, 2, xTq)]
                for is_q, nch, spc, xT in segs:
                    for ch in range(nch):
                        cs = slice(ch * 512, (ch + 1) * 512)
                        css = slice((ch % spc) * 512, (ch % spc + 1) * 512)
                        if ch % spc == 0:
                            hi8 = hlp.tile([128, 4, 2048], F8,
                                           tag="hi8", name=f"hi8{is_q}{ch}")
                            lo8 = hlp.tile([128, 4, 2048], F8,
                                           tag="lo8", name=f"lo8{is_q}{ch}")
                        if not is_q and ch == 3:
                            ensure_mask(0, 0)
                        if not is_q and ch == 4:
                            # aux k-row ones, issued mid-projection so the
                            # ACT queue serves it in an engine gap
                            for i in range(2):
                                nc.sync.dma_start(
                                    kdT8[i][64:65, 0].rearrange(
                                        "a h k -> a (h k)"), kaux_d[:])
                        # norms projection qn[8, 512] (bias via ACT)
                        pn = psN.tile([8, 512], F32, tag="pn")
                        for kc in range(2):
                            nc.tensor.matmul(
                                pn[:], w1t8[:, kc, :], xT[:, kc, cs],
                                start=(kc == 0), stop=(kc == 1))
                        if is_q:
                            # mq = SCALE*|qn| = |SCALE*pn + SCALE*b1|
                            nc.scalar.activation(
                                mq[:, cs], pn[:],
                                mybir.ActivationFunctionType.Abs,
                                bias=b18c[:, 1:2], scale=SCALE)
                        else:
                            sqn = smp.tile([8, 512], F32, tag="sqn")
                            nc.scalar.activation(
                                sqn[:], pn[:],
                                mybir.ActivationFunctionType.Square,
                                bias=b18c[:, 0:1], scale=1.0)
                            nc.vector.tensor_reduce(
                                sskp[:, ch:ch + 1], sqn[:],
                                axis=mybir.AxisListType.X,
                                op=mybir.AluOpType.add)
                        # per-group direction projections + scaling;
                        # rw = pr + b0 (an ACT/DVE op each — a fused variant
                        # reading pr and pe together is illegal: one PSUM
                        # input max per vector instruction, NCC_IBVF027)
                        sq_ = [None] * 4
                        rw_ = [None] * 4
                        for gp in range(4):
                            pr = psP.tile([128, 512], F32, tag="pr",
                                          name=f"pr{gp}")
                            for kc in range(2):
                                nc.tensor.matmul(
                                    pr[:], w0p[:, kc, gp, :], xT[:, kc, cs],
                                    start=(kc == 0), stop=(kc == 1))
                            sq_[gp] = sqp.tile([128, 512], BF16,
                                               tag=f"sq{gp}", name=f"sq{gp}")
                            nc.scalar.activation(
                                sq_[gp][:], pr[:],
                                mybir.ActivationFunctionType.Square,
                                bias=b0c[:, gp:gp + 1], scale=1.0)
                            rw_[gp] = sqp.tile([128, 512], F32,
                                               tag=f"rw{gp}", name=f"rw{gp}")
                            if gp < 2:
                                nc.scalar.activation(
                                    rw_[gp][:], pr[:],
                                    mybir.ActivationFunctionType.Identity,
                                    bias=b0c[:, gp:gp + 1], scale=1.0)
                            else:
                                nc.vector.tensor_scalar(
                                    out=rw_[gp][:], in0=pr[:],
                                    scalar1=b0c[:, gp:gp + 1], scalar2=0.0,
                                    op0=mybir.AluOpType.add,
                                    op1=mybir.AluOpType.add)
                        pss = psS.tile([8, 512], F32, tag="pss")
                        for gp in range(4):
                            nc.tensor.matmul(pss[:], inds[:, gp, :],
                                             sq_[gp][:],
                                             start=(gp == 0), stop=(gp == 3))
                        srt = smp.tile([8, 512], F32, tag="srt")
                        nc.scalar.activation(
                            srt[:], pss[:],
                            mybir.ActivationFunctionType.Sqrt,
                            scale=1.0 / (ALPHA * ALPHA))
                        rn = smp.tile([8, 512], F32, tag="rn")
                        nc.vector.reciprocal_approx_fast(rn[:], srt[:])
                        av = smp.tile([8, 512], F32, tag="av")
                        nc.vector.scalar_tensor_tensor(
                            out=av[:].bitcast(F32R), in0=pn[:],
                            scalar=b18c[:, 0:1], in1=rn[:],
                            op0=mybir.AluOpType.add,
                            op1=mybir.AluOpType.mult)
                        if is_q:
                            # aux8 = (B16 - A16*mq*tsh)/DIV, cast e4m3
                            nc.vector.tensor_scalar(
                                out=aux8[:, cs], in0=mq[:, cs],
                                scalar1=tshA[:], scalar2=B16 / DIV,
                                op0=mybir.AluOpType.mult,
                                op1=mybir.AluOpType.add)
                        xd = xdp.tile([128, 4, 512], BF16, tag="xd")
                        for gp in range(4):
                            pe = psE.tile([128, 512], F32, tag="pe")
                            mm(pe[:], indst[:, gp, :], av[:],
                               start=True, stop=True)
                            nc.vector.tensor_mul(
                                xd[:, gp], rw_[gp][:], pe[:])
                        # hi/lo e4m3 split on Pool
                        nc.gpsimd.tensor_scalar(
                            out=hi8[:, :, css], in0=xd[:], scalar1=1.0,
                            scalar2=0.0, op0=mybir.AluOpType.mult,
                            op1=mybir.AluOpType.add)
                        nc.gpsimd.tensor_tensor(
                            out=lo8[:, :, css], in0=xd[:],
                            in1=hi8[:, :, css],
                            op=mybir.AluOpType.subtract)
                        if ch % spc == spc - 1:
                            relayout(is_q, ch // spc)
                    if not is_q:
                        # shift scale between segments:
                        # tsh = LAM*SCALE^2*RMS(kn); tshA = -A16*tsh/DIV
                        nc.vector.tensor_reduce(ssk[:], sskp[:],
                                                axis=mybir.AxisListType.X,
                                                op=mybir.AluOpType.add)
                        nc.scalar.activation(
                            tsh[:], ssk[:],
                            mybir.ActivationFunctionType.Sqrt,
                            scale=LAM * LAM * SCALE * SCALE / float(SK))
                        nc.scalar.mul(tshA[:], tsh[:], -A16 / DIV)

            shp_ctx.__exit__(None, None, None)

            # ---- main attention loop ----
            with (
                tc.tile_pool(name="maskpB", bufs=1) as maskpB,
                tc.tile_pool(name="psSc", bufs=6, space="PSUM") as psc,
                tc.tile_pool(name="psNd", bufs=1, space="PSUM") as psnd,
                tc.tile_pool(name="eraw", bufs=6) as erawp,
                tc.tile_pool(name="etl", bufs=12) as etlp,
                tc.tile_pool(name="ndsb", bufs=2) as ndsbp,
            ):
                for _j in (1, 2, 3):
                    mask_pools[_j] = maskpB
                for ph, (rep, qh) in enumerate(phases):
                    for j in range(4):
                        ensure_mask(ph, j)
                    for ri, rot in enumerate(ROTS):
                        if ri % 2 == 0:
                            ndall = psnd.tile([98, 1024], F32, tag="ndall")
                        nb = 64 * (ri % 2)
                        last_rot = ri == len(ROTS) - 1
                        for kc in range(KT):
                            msl = mask_tiles[(ph, kc // 8)][:, kc % 8, :]
                            for slot, h in enumerate(rot):
                                co = nb + 32 * slot
                                kv = kdT8[kc // 16][
                                    0:65, :, h,
                                    (kc % 16) * 128:(kc % 16 + 1) * 128]
                                for j in range(2):
                                    js = slice(j * 512, (j + 1) * 512)
                                    ps = psc.tile([128, 512], F32, tag="ps",
                                                  name=f"ps{slot}{j}")
                                    qv = qdT8[qh][0:65, h,
                                                  j * 512:(j + 1) * 512]
                                    nc.tensor.matmul(
                                        ps[:], kv,
                                        qv.unsqueeze(1).broadcast_to(
                                            [65, 2, 512]),
                                        start=True, stop=True,
                                        perf_mode=(
                                            mybir.MatmulPerfMode.DoubleRow),
                                        tile_position=(0, 0))
                                    et = etlp.tile([128, 512], BF16,
                                                   tag="et")
                                    if ROUTE[(kc * 4 + slot * 2 + j)
                                             % 128] == 'd':
                                        # d: fused DVE schraudolph (u16 sat)
                                        nc.vector.scalar_tensor_tensor(
                                            out=et[:].bitcast(U16),
                                            in0=ps[:], scalar=DIV,
                                            in1=msl[:, js],
                                            op0=mybir.AluOpType.mult,
                                            op1=mybir.AluOpType.mult)
                                    else:
                                        # a: ACT exp -> Pool mask-mult
                                        er = erawp.tile([128, 512], BF16,
                                                        tag="er")
                                        nc.scalar.activation(
                                            er[:], ps[:],
                                            mybir.ActivationFunctionType.Exp,
                                            bias=expbias[:], scale=DIV / A16)
                                        nc.gpsimd.tensor_tensor(
                                            out=et[:], in0=er[:],
                                            in1=msl[:, js],
                                            op=mybir.AluOpType.mult)
                                    nc.tensor.matmul(
                                        ndall[co:co + 2,
                                              j * 512:(j + 1) * 512],
                                        uvt[:, kc, :], et[:],
                                        start=(kc == 0), stop=(kc == KT - 1),
                                        tile_position=(0, co))
                            if last_rot and kc % 8 == 7:
                                ensure_mask(ph + 1, kc // 8)
                        if ri % 2 == 1:
                            # one evacuation per rotation pair: the copy is
                            # charged by free size, so 98 rows cost the same
                            # as 34
                            ndsb = ndsbp.tile([98, 1024], F32, tag="ndsb")
                            nc.scalar.copy(ndsb[:], ndall[0:98, :])
                            for o, orot in enumerate(ROTS[ri - 1:ri + 1]):
                                for slot, h in enumerate(orot):
                                    r0 = 64 * o + 32 * slot
                                    nc.sync.dma_start(
                                        out_d[h:h + 1,
                                              qh * 2048:(qh + 1) * 2048],
                                        ndsb[r0:r0 + 2, :])

    nc.finalize()
    _CACHE[repeat] = nc
    return nc


def _prep_host(query, key, value, mask, w0, b0, w1, b1):
    import ml_dtypes
    # outc permutation: group gp = h//2 holds head 2gp at rows 0-31 and head
    # 2gp+1 at rows 64-95; rows 32-63/96-127 are zero padding.
    w0p = np.zeros((D, 4 * 128), np.float32)
    b0c = np.zeros((128, 4), np.float32)
    inds = np.zeros((128, 4 * H), np.float32)
    indst = np.zeros((H, 4 * 128), np.float32)
    w0t = w0.T.astype(np.float32)            # [inc, outc]
    for h in range(H):
        gp, u = divmod(h, 2)
        dst = gp * 128 + 64 * u
        w0p[:, dst:dst + 32] = w0t[:, 32 * h:32 * h + 32]
        b0c[64 * u:64 * u + 32, gp] = b0[32 * h:32 * h + 32]
        inds[64 * u:64 * u + 32, gp * H + h] = 1.0
        indst[h, gp * 128 + 64 * u:gp * 128 + 64 * u + 32] = 1.0
    w1t8 = np.ascontiguousarray(w1[:H].T).astype(ml_dtypes.bfloat16)
    b18c = np.stack([b1[:H], SCALE * b1[:H]], axis=1).astype(np.float32)
    w0p = w0p.astype(ml_dtypes.bfloat16)
    kaux = np.ones((1, H * SK // 2), ml_dtypes.float8_e4m3)
    in_maps = []
    for c in range(NCORES):
        b, half = divmod(c, 2)
        r0 = half * R
        mt = np.ascontiguousarray(mask[b, r0:r0 + R].T).astype(
            ml_dtypes.bfloat16)
        in_maps.append({
            "qT": np.ascontiguousarray(query[b, r0:r0 + R].T).astype(
                ml_dtypes.bfloat16),
            "kT": np.ascontiguousarray(key[b].T).astype(ml_dtypes.bfloat16),
            "v": np.ascontiguousarray(value[b].reshape(1, SK)),
            "mt": mt,
            "w0p": w0p, "w1t8": w1t8, "b0c": b0c, "b18c": b18c,
            "inds": inds, "indst": indst, "kaux": kaux,
        })
    return in_maps


def kernel(query, key, value, mask, w0, b0, w1, b1, _repeat=1):
    query = np.asarray(query, np.float32)
    key = np.asarray(key, np.float32)
    value = np.asarray(value, np.float32)
    mask = np.asarray(mask, np.int32)
    nc = _build(_repeat)
    in_maps = _prep_host(query, key, value, mask, w0, b0, w1, b1)
    res = bass_utils.run_bass_kernel_spmd(nc, in_maps,
                                          core_ids=list(range(NCORES)))
    out = np.empty((B, SQ, 1), np.float32)
    for c in range(NCORES):
        b, half = divmod(c, 2)
        o = res.results[c]["o"].reshape(8, QH, 2048)
        x = o[:, :, 0:1024] / o[:, :, 1024:2048]
        out[b, half * R:(half + 1) * R, 0] = x.reshape(8, R).mean(axis=0)
    return out
